# revision 24
# baseline (speedup 1.0000x reference)
"""Trainium2 Bass kernel for nn_LocalConnectivity (diamond-ring circular stencil).

out[i,j] = sum_{d=1..5} w_d * sum_{|di|+|dj|=d} x[(i+di)%H, (j+dj)%W]

Strategy: row-shard across 8 NeuronCores (512 rows each + 5-row circular
halo, columns pre-padded with 5-col circular halo on host), all in bf16.
The 11x11 kernel K[di,dj] is symmetric in dj, so columns +dj and -dj share
one vertical band: DVE pre-sums the +-dj shifted slices (jj-major, 1024-col
blocks) and the TensorEngine applies 6 banded matmuls per 512-col chunk,
jj-major across all 8 PSUM banks so consecutive LDWEIGHTS reuse the same
stationary band. All strips prefetch up front (bufs=5). DRAM-side APs of
every DMA are strided (column-split) so each row is its own descriptor
run -- a fully contiguous transfer becomes ONE run pinned to ONE ~22 GB/s
DMA engine, while strided runs round-robin across all 16.
"""
import numpy as np
from contextlib import ExitStack

import concourse.bass as bass
import concourse.tile as tile
from concourse import bacc, mybir
from concourse.bass_utils import run_bass_kernel_spmd

try:
    import ml_dtypes
    _BF16 = np.dtype(ml_dtypes.bfloat16)
except ImportError:  # pragma: no cover
    _BF16 = None

N_CORES = 8
H = W = 4096
MAXD = 5
ROWS_PER_CORE = H // N_CORES          # 512
IN_ROWS = ROWS_PER_CORE + 2 * MAXD    # 522
IN_COLS = W + 2 * MAXD                # 4106
NCOL = 512                            # matmul free dim (one PSUM bank, fp32 max)
NCHUNK = W // NCOL                    # 8
M_OUT = 118                           # output rows per row-window (K=128 - 2*MAXD)
NBAND = MAXD + 1                      # 6 vertical bands (|dj| = 0..5)
BLK = 2048                            # pair-sum emission granularity (cols)
# row windows: (input_row_start, out_row_start, K, M). The small remainder
# window goes FIRST: its strip is the smallest load, so the PE starts sooner.
WINDOWS = [(0, 0, (ROWS_PER_CORE % M_OUT) + 2 * MAXD, ROWS_PER_CORE % M_OUT)]
_o = ROWS_PER_CORE % M_OUT
while _o < ROWS_PER_CORE:
    WINDOWS.append((_o, _o, M_OUT + 2 * MAXD, M_OUT))
    _o += M_OUT

_CACHE = {}


def _band_weights6(distance_weights: np.ndarray) -> np.ndarray:
    """w6 [128, 6*118]: w6[k, jj*118 + m] = K2d[k-m-5, jj] for |dj|=jj."""
    wd = np.asarray(distance_weights, dtype=np.float32)
    w = np.zeros((NBAND, 128, M_OUT), dtype=np.float32)
    for jj in range(NBAND):  # |dj| = 0..5
        for di in range(-MAXD, MAXD + 1):
            d = abs(di) + jj
            if not (1 <= d <= MAXD):
                continue
            m = np.arange(M_OUT)
            k = m + MAXD + di
            ok = (k >= 0) & (k < 128)
            w[jj, k[ok], m[ok]] = wd[d - 1]
    w = np.ascontiguousarray(w.transpose(1, 0, 2).reshape(128, NBAND * M_OUT))
    return w.astype(_BF16)


def _build():
    dtb = mybir.dt.bfloat16
    nc = bacc.Bacc("TRN2", target_bir_lowering=False, debug=False,
                   num_devices=N_CORES)
    x = nc.dram_tensor("x", [IN_ROWS, IN_COLS], dtb, kind="ExternalInput").ap()
    wts = nc.dram_tensor("w", [128, NBAND * M_OUT], dtb,
                         kind="ExternalInput").ap()
    y = nc.dram_tensor("y", [ROWS_PER_CORE, W], dtb, kind="ExternalOutput").ap()

    with tile.TileContext(nc) as tc, ExitStack() as ctx:
        spool = ctx.enter_context(tc.tile_pool(name="strip", bufs=5))
        wpool = ctx.enter_context(tc.tile_pool(name="wts", bufs=1))
        mpool = ctx.enter_context(tc.tile_pool(name="sums", bufs=3))
        opool = ctx.enter_context(tc.tile_pool(name="out", bufs=2))
        ppool = ctx.enter_context(tc.tile_pool(name="ps", bufs=8, space="PSUM"))

        CMID = IN_COLS // 2
        strips = []

        def load_strip(in0, kdim):
            # Column-split halves: DRAM runs shorter than the row stride, so
            # each row is its own descriptor and fans across all DMA engines.
            st = spool.tile([128, IN_COLS], dtb, tag="strip")
            nc.gpsimd.dma_start(st[:kdim, :CMID], x[in0:in0 + kdim, :CMID])
            nc.scalar.dma_start(st[:kdim, CMID:], x[in0:in0 + kdim, CMID:])
            return st

        # Weights first on the otherwise-idle sync queue (tiny, gates the
        # first LDWEIGHTS), then prefetch ALL strips (they fit in SBUF).
        # The first (smallest) strip is 3-way split so it lands fastest.
        wt = wpool.tile([128, NBAND * M_OUT], dtb)
        WMID = (NBAND * M_OUT) // 2
        nc.sync.dma_start(wt[:, :WMID], wts[:, :WMID])
        nc.sync.dma_start(wt[:, WMID:], wts[:, WMID:])
        st0 = spool.tile([128, IN_COLS], dtb, tag="strip", name="strip0")
        kdim0 = WINDOWS[0][2]
        CQ = IN_COLS // 3
        for qi, eng in enumerate((nc.gpsimd, nc.scalar, nc.sync)):
            cl, cr = qi * CQ, IN_COLS if qi == 2 else (qi + 1) * CQ
            eng.dma_start(st0[:kdim0, cl:cr], x[0:kdim0, cl:cr])
        strips.append(st0)
        for (in0, _o0, kdim, _m) in WINDOWS[1:]:
            strips.append(load_strip(in0, kdim))

        for wi, (in0, out0, kdim, m) in enumerate(WINDOWS):
            st = strips[wi]

            # Paired column sums on DVE, jj-major in 1024-col blocks so each
            # band's sum stream stays ahead of the PE's same-band pass:
            # s_jj[:, j] = x[:, j+5-jj] + x[:, j+5+jj]
            sm = mpool.tile([128, MAXD * W], dtb, tag="sums")
            for jj in range(1, MAXD + 1):
                for b0 in range(0, W, BLK):
                    nc.vector.tensor_add(
                        sm[:kdim, (jj - 1) * W + b0:(jj - 1) * W + b0 + BLK],
                        st[:kdim, MAXD - jj + b0:MAXD - jj + b0 + BLK],
                        st[:kdim, MAXD + jj + b0:MAXD + jj + b0 + BLK],
                    )

            # jj-major matmul passes: one stationary band applied to all 8
            # chunks (one PSUM bank each) before switching bands, so
            # consecutive LDWEIGHTS hit the same stationary.
            ot = opool.tile([m, W], dtb, tag="out")
            pss = [ppool.tile([m, NCOL], mybir.dt.float32, tag="ps",
                              name=f"ps_w{wi}c{cc}")
                   for cc in range(NCHUNK)]
            last = NBAND - 1
            # Store in 2-chunk pieces throughout: keeps the volume of
            # in-flight stores small (cheap mid-kernel DMA-drain barrier,
            # short kernel tail).
            sgran = 2 * NCOL
            for jj in range(NBAND):
                for cc in range(NCHUNK):
                    c0 = cc * NCOL
                    if jj == 0:
                        rhs = st[:kdim, MAXD + c0:MAXD + c0 + NCOL]
                    else:
                        rhs = sm[:kdim, (jj - 1) * W + c0:(jj - 1) * W + c0 + NCOL]
                    nc.tensor.matmul(
                        pss[cc], wt[:kdim, jj * M_OUT:jj * M_OUT + m], rhs,
                        start=(jj == 0), stop=(jj == last),
                    )
                    if jj == last:
                        # Drain inline while the PE streams the next chunk.
                        nc.scalar.copy(ot[:, c0:c0 + NCOL], pss[cc])
                        # Store as soon as a piece is fully drained; pieces
                        # split column-wise across two queues so DRAM runs
                        # stay strided (per-row descriptors -> all 16 DMA
                        # engines). Keep stores OFF the scalar queue -- its
                        # HWDGE is driven by the Act sequencer, which is
                        # busy draining PSUM.
                        if (c0 + NCOL) % sgran == 0:
                            h0, q = c0 + NCOL - sgran, sgran // 2
                            nc.sync.dma_start(y[out0:out0 + m, h0:h0 + q],
                                              ot[:, h0:h0 + q])
                            nc.gpsimd.dma_start(
                                y[out0:out0 + m, h0 + q:h0 + 2 * q],
                                ot[:, h0 + q:h0 + 2 * q])
    nc.compile()
    return nc


def _in_maps(grid_spikes: np.ndarray, distance_weights: np.ndarray):
    x = np.ascontiguousarray(grid_spikes, dtype=np.float32)
    assert x.shape == (H, W)
    w6 = _band_weights6(distance_weights)
    xpad = np.concatenate([x[:, -MAXD:], x, x[:, :MAXD]], axis=1).astype(_BF16)
    in_maps = []
    for c in range(N_CORES):
        rows = np.arange(c * ROWS_PER_CORE - MAXD,
                         c * ROWS_PER_CORE + ROWS_PER_CORE + MAXD) % H
        in_maps.append({"x": np.ascontiguousarray(xpad[rows]), "w": w6})
    return in_maps


def kernel(grid_spikes: np.ndarray, distance_weights: np.ndarray) -> np.ndarray:
    if "nc" not in _CACHE:
        _CACHE["nc"] = _build()
    nc = _CACHE["nc"]
    in_maps = _in_maps(grid_spikes, distance_weights)
    res = run_bass_kernel_spmd(nc, in_maps, list(range(N_CORES)))
    out = np.concatenate(
        [np.asarray(res.results[c]["y"]) for c in range(N_CORES)], axis=0)
    return out.astype(np.float32)


# revision 25
# speedup vs baseline: 1.0303x; 1.0303x over previous
"""Trainium2 Bass kernel for nn_LocalConnectivity (diamond-ring circular stencil).

out[i,j] = sum_{d=1..5} w_d * sum_{|di|+|dj|=d} x[(i+di)%H, (j+dj)%W]

Strategy: row-shard across 8 NeuronCores (512 rows each + 5-row circular
halo, columns pre-padded with 5-col circular halo on host), all in bf16.
The 11x11 kernel K[di,dj] is symmetric in dj, so columns +dj and -dj share
one vertical band: DVE pre-sums the +-dj shifted slices (jj-major, 1024-col
blocks) and the TensorEngine applies 6 banded matmuls per 512-col chunk,
jj-major across all 8 PSUM banks so consecutive LDWEIGHTS reuse the same
stationary band. All strips prefetch up front (bufs=5). DRAM-side APs of
every DMA are strided (column-split) so each row is its own descriptor
run -- a fully contiguous transfer becomes ONE run pinned to ONE ~22 GB/s
DMA engine, while strided runs round-robin across all 16.
"""
import numpy as np
from contextlib import ExitStack

import concourse.bass as bass
import concourse.tile as tile
from concourse import bacc, mybir
from concourse.bass_utils import run_bass_kernel_spmd

try:
    import ml_dtypes
    _BF16 = np.dtype(ml_dtypes.bfloat16)
except ImportError:  # pragma: no cover
    _BF16 = None

N_CORES = 8
H = W = 4096
MAXD = 5
ROWS_PER_CORE = H // N_CORES          # 512
IN_ROWS = ROWS_PER_CORE + 2 * MAXD    # 522
IN_COLS = W + 2 * MAXD                # 4106
NCOL = 512                            # matmul free dim (one PSUM bank, fp32 max)
NCHUNK = W // NCOL                    # 8
M_OUT = 118                           # output rows per row-window (K=128 - 2*MAXD)
NBAND = MAXD + 1                      # 6 vertical bands (|dj| = 0..5)
BLK = 2048                            # pair-sum emission granularity (cols)
# row windows: (input_row_start, out_row_start, K, M). The small remainder
# window goes FIRST: its strip is the smallest load, so the PE starts sooner.
WINDOWS = [(0, 0, (ROWS_PER_CORE % M_OUT) + 2 * MAXD, ROWS_PER_CORE % M_OUT)]
_o = ROWS_PER_CORE % M_OUT
while _o < ROWS_PER_CORE:
    WINDOWS.append((_o, _o, M_OUT + 2 * MAXD, M_OUT))
    _o += M_OUT

_CACHE = {}


def _band_weights6(distance_weights: np.ndarray) -> np.ndarray:
    """w6 [128, 6*118]: w6[k, jj*118 + m] = K2d[k-m-5, jj] for |dj|=jj."""
    wd = np.asarray(distance_weights, dtype=np.float32)
    w = np.zeros((NBAND, 128, M_OUT), dtype=np.float32)
    for jj in range(NBAND):  # |dj| = 0..5
        for di in range(-MAXD, MAXD + 1):
            d = abs(di) + jj
            if not (1 <= d <= MAXD):
                continue
            m = np.arange(M_OUT)
            k = m + MAXD + di
            ok = (k >= 0) & (k < 128)
            w[jj, k[ok], m[ok]] = wd[d - 1]
    w = np.ascontiguousarray(w.transpose(1, 0, 2).reshape(128, NBAND * M_OUT))
    return w.astype(_BF16)


def _build():
    dtb = mybir.dt.bfloat16
    nc = bacc.Bacc("TRN2", target_bir_lowering=False, debug=False,
                   num_devices=N_CORES)
    x = nc.dram_tensor("x", [IN_ROWS, IN_COLS], dtb, kind="ExternalInput").ap()
    wts = nc.dram_tensor("w", [128, NBAND * M_OUT], dtb,
                         kind="ExternalInput").ap()
    y = nc.dram_tensor("y", [ROWS_PER_CORE, W], dtb, kind="ExternalOutput").ap()

    with tile.TileContext(nc) as tc, ExitStack() as ctx:
        spool = ctx.enter_context(tc.tile_pool(name="strip", bufs=5))
        wpool = ctx.enter_context(tc.tile_pool(name="wts", bufs=1))
        mpool = ctx.enter_context(tc.tile_pool(name="sums", bufs=3))
        opool = ctx.enter_context(tc.tile_pool(name="out", bufs=2))
        ppool = ctx.enter_context(tc.tile_pool(name="ps", bufs=8, space="PSUM"))

        CMID = IN_COLS // 2
        strips = []

        def load_strip(in0, kdim):
            # Column-split halves: DRAM runs shorter than the row stride, so
            # each row is its own descriptor and fans across all DMA engines.
            st = spool.tile([128, IN_COLS], dtb, tag="strip")
            nc.gpsimd.dma_start(st[:kdim, :CMID], x[in0:in0 + kdim, :CMID])
            nc.scalar.dma_start(st[:kdim, CMID:], x[in0:in0 + kdim, CMID:])
            return st

        # Weights first on the otherwise-idle sync queue (tiny, gates the
        # first LDWEIGHTS), then prefetch ALL strips (they fit in SBUF).
        # The first (smallest) strip is 3-way split so it lands fastest.
        wt = wpool.tile([128, NBAND * M_OUT], dtb)
        WMID = (NBAND * M_OUT) // 2
        nc.sync.dma_start(wt[:, :WMID], wts[:, :WMID])
        nc.sync.dma_start(wt[:, WMID:], wts[:, WMID:])
        st0 = spool.tile([128, IN_COLS], dtb, tag="strip", name="strip0")
        kdim0 = WINDOWS[0][2]
        CQ = IN_COLS // 3
        for qi, eng in enumerate((nc.gpsimd, nc.scalar, nc.sync)):
            cl, cr = qi * CQ, IN_COLS if qi == 2 else (qi + 1) * CQ
            eng.dma_start(st0[:kdim0, cl:cr], x[0:kdim0, cl:cr])
        strips.append(st0)
        for (in0, _o0, kdim, _m) in WINDOWS[1:]:
            strips.append(load_strip(in0, kdim))

        for wi, (in0, out0, kdim, m) in enumerate(WINDOWS):
            st = strips[wi]

            # Paired column sums on DVE, jj-major in 1024-col blocks so each
            # band's sum stream stays ahead of the PE's same-band pass:
            # s_jj[:, j] = x[:, j+5-jj] + x[:, j+5+jj]
            sm = mpool.tile([128, MAXD * W], dtb, tag="sums")
            for jj in range(1, MAXD + 1):
                for b0 in range(0, W, BLK):
                    nc.vector.tensor_add(
                        sm[:kdim, (jj - 1) * W + b0:(jj - 1) * W + b0 + BLK],
                        st[:kdim, MAXD - jj + b0:MAXD - jj + b0 + BLK],
                        st[:kdim, MAXD + jj + b0:MAXD + jj + b0 + BLK],
                    )

            # jj-major matmul passes: one stationary band applied to all 8
            # chunks (one PSUM bank each) before switching bands, so
            # consecutive LDWEIGHTS hit the same stationary.
            ot = opool.tile([m, W], dtb, tag="out")
            pss = [ppool.tile([m, NCOL], mybir.dt.float32, tag="ps",
                              name=f"ps_w{wi}c{cc}")
                   for cc in range(NCHUNK)]
            last = NBAND - 1
            # Store granularity: the final window streams out in 2-chunk
            # pieces so the kernel tail is only one small store deep; other
            # windows store half-window pieces (SWDGE setup is ~1us/start,
            # so more starts on the gpsimd queue backfire).
            sgran = 2 * NCOL if wi == len(WINDOWS) - 1 else W // 2
            for jj in range(NBAND):
                for cc in range(NCHUNK):
                    c0 = cc * NCOL
                    if jj == 0:
                        rhs = st[:kdim, MAXD + c0:MAXD + c0 + NCOL]
                    else:
                        rhs = sm[:kdim, (jj - 1) * W + c0:(jj - 1) * W + c0 + NCOL]
                    nc.tensor.matmul(
                        pss[cc], wt[:kdim, jj * M_OUT:jj * M_OUT + m], rhs,
                        start=(jj == 0), stop=(jj == last),
                    )
                    if jj == last:
                        # Drain inline while the PE streams the next chunk.
                        nc.scalar.copy(ot[:, c0:c0 + NCOL], pss[cc])
                        # Store as soon as a piece is fully drained; pieces
                        # split column-wise across two queues so DRAM runs
                        # stay strided (per-row descriptors -> all 16 DMA
                        # engines). Keep stores OFF the scalar queue -- its
                        # HWDGE is driven by the Act sequencer, which is
                        # busy draining PSUM.
                        if (c0 + NCOL) % sgran == 0:
                            h0, q = c0 + NCOL - sgran, sgran // 2
                            nc.sync.dma_start(y[out0:out0 + m, h0:h0 + q],
                                              ot[:, h0:h0 + q])
                            nc.gpsimd.dma_start(
                                y[out0:out0 + m, h0 + q:h0 + 2 * q],
                                ot[:, h0 + q:h0 + 2 * q])
    nc.compile()
    return nc


def _in_maps(grid_spikes: np.ndarray, distance_weights: np.ndarray):
    x = np.ascontiguousarray(grid_spikes, dtype=np.float32)
    assert x.shape == (H, W)
    w6 = _band_weights6(distance_weights)
    xpad = np.concatenate([x[:, -MAXD:], x, x[:, :MAXD]], axis=1).astype(_BF16)
    in_maps = []
    for c in range(N_CORES):
        rows = np.arange(c * ROWS_PER_CORE - MAXD,
                         c * ROWS_PER_CORE + ROWS_PER_CORE + MAXD) % H
        in_maps.append({"x": np.ascontiguousarray(xpad[rows]), "w": w6})
    return in_maps


def kernel(grid_spikes: np.ndarray, distance_weights: np.ndarray) -> np.ndarray:
    if "nc" not in _CACHE:
        _CACHE["nc"] = _build()
    nc = _CACHE["nc"]
    in_maps = _in_maps(grid_spikes, distance_weights)
    res = run_bass_kernel_spmd(nc, in_maps, list(range(N_CORES)))
    out = np.concatenate(
        [np.asarray(res.results[c]["y"]) for c in range(N_CORES)], axis=0)
    return out.astype(np.float32)
